# revision 3
# baseline (speedup 1.0000x reference)
"""Trainium2 Bass kernel for nn_MultiHeadCrossAttention (BS=4, S=512, DM=512, H=8).

Sharding: one attention head per NeuronCore (8 heads / 8 cores). Each core
receives the full (transposed) q/k/v plus its head's weight slices, computes
its head end-to-end including the rank-64 slice of the output projection, and
the host sums the 8 partial outputs.

v2 restructuring vs v1:
  - E^T matmuls (K=64) row-tiled: kb pairs run concurrently on PE row halves
    (khT stored split across partition halves, qhT duplicated to both halves).
  - Projections col-tiled into partition halves matching their SBUF targets.
  - One exp activation per (b, jc) over [128, 4, 512] PSUM (4 banks).
  - P6 packed to K=128: out = [Wo; 4*Wo]^T @ [heads; qhT].
  - e2 = e*e on DVE instead of a second ACT exp pass; e stored bf16.
  - rt-mult and P5 elementwise offloaded to GPSIMD; P6 output copies split
    DVE (b=0,1) / ACT (b=2,3, tail when the exp stream is done).
  - PE warmup matmuls at t0 (HAM clock-gate) + early Exp table load.
"""

import numpy as np

BS, S, DM, H, DK = 4, 512, 512, 8, 64
EPS = 1e-6
NCORES = 8


def build_program(nc, tile, mybir):
    f32 = mybir.dt.float32
    bf16 = mybir.dt.bfloat16
    i32 = mybir.dt.int32
    AF = mybir.ActivationFunctionType
    OP = mybir.AluOpType
    AX = mybir.AxisListType

    # ---- DRAM I/O ----
    qT_d = nc.dram_tensor("qT", [BS, 128, 4, S], bf16, kind="ExternalInput")
    kT_d = nc.dram_tensor("kT", [BS, 128, 4, S], bf16, kind="ExternalInput")
    vT_d = nc.dram_tensor("vT", [BS, 128, 4, S], bf16, kind="ExternalInput")
    Wq_d = nc.dram_tensor("Wq", [128, 4, DK], bf16, kind="ExternalInput")
    Wk_d = nc.dram_tensor("Wk", [128, 4, DK], bf16, kind="ExternalInput")
    Wv_d = nc.dram_tensor("Wv", [128, 4, DK], bf16, kind="ExternalInput")
    bqc_d = nc.dram_tensor("bqc", [DK, 1], f32, kind="ExternalInput")
    bkc_d = nc.dram_tensor("bkc", [DK, 1], f32, kind="ExternalInput")
    bv_d = nc.dram_tensor("bv", [1, DK], bf16, kind="ExternalInput")
    WoP_d = nc.dram_tensor("WoP", [128, 4, 128], bf16, kind="ExternalInput")
    bo2_d = nc.dram_tensor("bo2", [128, 4], f32, kind="ExternalInput")
    al_d = nc.dram_tensor("alpha", [DK, 1], f32, kind="ExternalInput")
    b4_d = nc.dram_tensor("beta4", [DK, 1], f32, kind="ExternalInput")
    id_d = nc.dram_tensor("ident", [128, 128], bf16, kind="ExternalInput")
    idf_d = nc.dram_tensor("identf", [128, 128], f32, kind="ExternalInput")
    outT_d = nc.dram_tensor("outT", [BS, DM, S], bf16, kind="ExternalOutput")

    with tile.TileContext(nc) as tc:
        with (
            tc.tile_pool(name="persist", bufs=1) as pp,
            tc.tile_pool(name="consts", bufs=1) as cp,
            tc.tile_pool(name="kin", bufs=1) as kip,
            tc.tile_pool(name="vin", bufs=1) as vip,
            tc.tile_pool(name="qin", bufs=2) as qip,
            tc.tile_pool(name="exw", bufs=3) as exp_pool,
            tc.tile_pool(name="wrw", bufs=2) as wrp,
            tc.tile_pool(name="p5w", bufs=2) as p5p,
            tc.tile_pool(name="otw", bufs=3) as otp,
            tc.tile_pool(name="stats", bufs=2) as stp,
            tc.tile_pool(name="psum", bufs=1, space="PSUM") as psp,
        ):
            # ---- persistent SBUF ----
            # qq: qhT duplicated in both partition halves  [d(2x64), b, i]
            qq = pp.tile([128, BS, S], bf16, tag="qq")
            # hq: top = heads (P5 output), bottom = qhT    (P6 rhs)
            hq = pp.tile([128, BS, S], bf16, tag="hq")
            # khT2: slot s: [64:128] = kb 2s (even), [0:64] = kb 2s+1 (odd)
            khT2 = pp.tile([128, 2, S], bf16, tag="khT2")
            vh_all = pp.tile([128, 4, BS, DK], bf16, tag="vh")   # [j, jc, c, d]
            rt_all = pp.tile([128, BS, 4, S], bf16, tag="rt")    # [j, b, jc, i]
            # e_all: [i, b, u, (ic c d)]  u=0: e=exp(s), u=1: e2=e*e
            e_all = pp.tile([128, BS, 2, 1024], bf16, tag="e")
            Z_all = pp.tile([128, 64], f32, tag="Z")   # cols = b*16 + ic*4 + c
            Q_all = pp.tile([128, 64], bf16, tag="Q")
            w1_all = pp.tile([128, 64], f32, tag="w1")
            w0_all = pp.tile([128, 16], f32, tag="w0")  # cols = b*4 + ic
            w0T0 = pp.tile([8, 128], f32, tag="w0T0")
            w0T1 = pp.tile([8, 128], f32, tag="w0T1")
            w0f = pp.tile([1, 16 * 128], f32, tag="w0f")
            bvb = pp.tile([128, DK], bf16, tag="bvb")

            Wq_s = cp.tile([128, 4, DK], bf16, tag="Wq")
            Wk_s = cp.tile([128, 4, DK], bf16, tag="Wk")
            Wv_s = cp.tile([128, 4, DK], bf16, tag="Wv")
            WoP_s = cp.tile([128, 4, 128], bf16, tag="WoP")
            bo2_s = cp.tile([128, 4], f32, tag="bo2")
            bqc_s = cp.tile([DK, 1], f32, tag="bqc")
            bkc_s = cp.tile([DK, 1], f32, tag="bkc")
            bv_s = cp.tile([1, DK], bf16, tag="bv")
            al_s = cp.tile([DK, 1], f32, tag="al")
            b4_s = cp.tile([DK, 1], f32, tag="b4")
            id_s = cp.tile([128, 128], bf16, tag="id")
            idf_s = cp.tile([128, 128], f32, tag="idf")
            ones_b = cp.tile([1, 128], bf16, tag="ones_b")
            ones_f = cp.tile([1, 128], f32, tag="ones_f")
            warm_z = cp.tile([128, S], bf16, tag="warm_z")

            # ---- DMA: weights first (small), then kT all, then qT/vT
            # interleaved.  sync queue is FIFO: order = arrival order.
            nc.sync.dma_start(id_s[:], id_d[:])
            nc.sync.dma_start(Wk_s[:], Wk_d[:])
            nc.sync.dma_start(Wq_s[:], Wq_d[:])
            nc.sync.dma_start(Wv_s[:], Wv_d[:])
            nc.sync.dma_start(bqc_s[:], bqc_d[:])
            nc.sync.dma_start(bkc_s[:], bkc_d[:])
            nc.sync.dma_start(bv_s[:], bv_d[:])
            nc.sync.dma_start(idf_s[:], idf_d[:])
            nc.sync.dma_start(WoP_s[:], WoP_d[:])
            nc.sync.dma_start(bo2_s[:], bo2_d[:])
            nc.sync.dma_start(al_s[:], al_d[:])
            nc.sync.dma_start(b4_s[:], b4_d[:])
            ktiles, qtiles, vtiles = [], [None] * BS, [None] * BS
            for b in range(BS):
                kt = kip.tile([128, 4, S], bf16, tag=f"kt{b}")
                nc.sync.dma_start(kt[:], kT_d[b])
                ktiles.append(kt)
            for b in range(BS):
                qt = qip.tile([128, 4, S], bf16, tag="qt", name=f"qt{b}")
                nc.sync.dma_start(qt[:], qT_d[b])
                qtiles[b] = qt
                vt = vip.tile([128, 4, S], bf16, tag=f"vt{b}")
                nc.sync.dma_start(vt[:], vT_d[b])
                vtiles[b] = vt

            nc.vector.memset(ones_b[:], 1.0)
            nc.vector.memset(ones_f[:], 1.0)
            nc.vector.memset(warm_z[:], 0.0)

            # ---- PSUM tags: pe [128,4,512] = 4 banks; su x2 = 2; sc x2 = 2
            def psum_su():
                return psp.tile([128, S], f32, tag="su", bufs=2, name="psu")

            def psum_sc():
                return psp.tile([128, S], f32, tag="sc", bufs=2, name="psc")

            def psum_pe():
                return psp.tile([128, 4, S], f32, tag="pe", bufs=1, name="ppe")

            # ---- PE warmup (HAM clock-gate): ~8 N=512 matmuls on zeros ----
            wps = psum_pe()
            for w in range(8):
                nc.tensor.matmul(wps[:, w % 4, :], id_s[:], warm_z[:],
                                 start=True, stop=True)

            # ---- P1: projections ----
            # k-proj kb: even kb -> psum[64:128] -> khT2[64:128, kb//2]
            #            odd  kb -> psum[0:64]   -> khT2[0:64,  kb//2]
            def emit_kproj(kb):
                ps = psum_su() if kb % 2 == 0 else psum_sc()
                half = slice(64, 128) if kb % 2 == 0 else slice(0, 64)
                for mc in range(4):
                    nc.tensor.matmul(ps[half, :], Wk_s[:, mc, :],
                                     ktiles[kb][:, mc, :],
                                     start=(mc == 0), stop=(mc == 3))
                nc.scalar.activation(khT2[half, kb // 2, :], ps[half, :],
                                     AF.Identity, bias=bkc_s[:])

            # q-proj b: b 0,2 -> psum[0:64] -> qq[0:64, b];  b 1,3 -> bottom.
            # Then gpsimd-queue DMAs duplicate into the other half + hq bottom.
            def emit_qproj(b):
                ps = psum_su() if b % 2 == 0 else psum_sc()
                half = slice(0, 64) if b % 2 == 0 else slice(64, 128)
                for mc in range(4):
                    nc.tensor.matmul(ps[half, :], Wq_s[:, mc, :],
                                     qtiles[b][:, mc, :],
                                     start=(mc == 0), stop=(mc == 3))
                nc.scalar.activation(qq[half, b, :], ps[half, :],
                                     AF.Identity, bias=bqc_s[:])
                other = slice(64, 128) if b % 2 == 0 else slice(0, 64)
                nc.gpsimd.dma_start(qq[other, b, :], qq[half, b, :])
                nc.gpsimd.dma_start(hq[64:128, b, :], qq[half, b, :])

            # ---- bvb: broadcast bv across partitions via K=1 matmul ----
            def emit_bvb():
                pb = psum_sc()
                nc.tensor.matmul(pb[:, 0:DK], ones_b[:, :], bv_s[:],
                                 start=True, stop=True)
                nc.vector.tensor_copy(bvb[:], pb[:, 0:DK])

            # ---- vh per c: vh_all[j, jc, c, d] = (vt @ Wv) + bv ----
            def emit_vh(c):
                vt = vtiles[c]
                pv = psum_sc()
                for jc in range(4):
                    for mc in range(4):
                        nc.tensor.matmul(
                            pv[:, jc * DK:(jc + 1) * DK],
                            vt[:, mc, jc * 128:(jc + 1) * 128],
                            Wv_s[:, mc, :],
                            start=(mc == 0), stop=(mc == 3),
                        )
                nc.vector.tensor_tensor(
                    vh_all[:, :, c, :],
                    pv[:, 0:4 * DK].rearrange("p (jc d) -> p jc d", d=DK),
                    bvb[:].unsqueeze(1).broadcast_to((128, 4, DK)),
                    op=OP.add)

            # ---- P2: E^T + fenmu + rt, per (b, jc) ----
            def emit_e(b, jc):
                ph = psum_pe()
                js = slice(jc * 128, (jc + 1) * 128)
                # kb pairs (even bottom rows, odd top rows) run concurrently
                for s_ in range(2):
                    nc.tensor.matmul(ph[:, 2 * s_, :], khT2[64:128, s_, js],
                                     qq[64:128, b, :], start=True, stop=True)
                    nc.tensor.matmul(ph[:, 2 * s_ + 1, :], khT2[0:64, s_, js],
                                     qq[0:64, b, :], start=True, stop=True)
                ex = exp_pool.tile([128, 4, S], bf16, tag="ex")
                nc.scalar.activation(ex[:], ph[:], AF.Exp)
                return ex

            def emit_fenmu_rt(b, jc, ex, rt_eng):
                su = psum_su()
                for kb in range(4):
                    nc.tensor.matmul(su[:], id_s[:], ex[:, kb, :],
                                     start=(kb == 0), stop=(kb == 3))
                wr = wrp.tile([128, S], f32, tag="wr")
                nc.vector.reciprocal_approx_fast(wr[:], su[:])
                rt_eng.tensor_tensor(rt_all[:, b, jc, :], ex[:, b, :], wr[:],
                                     op=OP.mult)

            def emit_p2(b, rt_gp):
                for jc in range(4):
                    ex = emit_e(b, jc)
                    eng = nc.gpsimd if rt_gp[jc] else nc.vector
                    emit_fenmu_rt(b, jc, ex, eng)

            # ---- P3: score + e/e2 + Z/Q, per b ----
            def emit_p3(b):
                for p in range(2):
                    sc_ = psum_sc()
                    for i2 in range(2):
                        ic = 2 * p + i2
                        for jc in range(4):
                            nc.tensor.matmul(
                                sc_[:, i2 * 256:i2 * 256 + BS * DK],
                                rt_all[:, b, jc, ic * 128:(ic + 1) * 128],
                                vh_all[:, jc].rearrange("p c d -> p (c d)"),
                                start=(jc == 0), stop=(jc == 3),
                            )
                    nc.scalar.activation(
                        e_all[:, b, 0, p * 512:(p + 1) * 512], sc_[:], AF.Exp)
                nc.vector.tensor_tensor(e_all[:, b, 1, :], e_all[:, b, 0, :],
                                        e_all[:, b, 0, :], op=OP.mult)
                nc.vector.tensor_reduce(
                    Z_all[:, b * 16:(b + 1) * 16],
                    e_all[:, b, 0, :].rearrange("p (g d) -> p g d", d=DK),
                    axis=AX.X, op=OP.add)
                with nc.allow_low_precision("Q moment tolerates bf16"):
                    nc.vector.tensor_reduce(
                        Q_all[:, b * 16:(b + 1) * 16],
                        e_all[:, b, 1, :].rearrange("p (g d) -> p g d", d=DK),
                        axis=AX.X, op=OP.add)

            # ---- P4: stats per b-pair h (w1, w0) — DVE bit-trick rsqrt ----
            def emit_stats(h):
                c0, c1 = h * 32, (h + 1) * 32
                Zs, Qs = Z_all[:, c0:c1], Q_all[:, c0:c1]
                t = stp.tile([128, 32], f32, tag="t", name="t")
                nc.vector.tensor_tensor(t[:], Zs, Zs, op=OP.mult)
                s = stp.tile([128, 32], f32, tag="s", name="s")
                nc.vector.scalar_tensor_tensor(
                    s[:], t[:], -1.0 / DK, Qs, op0=OP.mult, op1=OP.add)
                rinv = stp.tile([128, 32], f32, tag="rinv", name="rinv")
                nc.vector.reciprocal(rinv[:], t[:])
                v63 = stp.tile([128, 32], f32, tag="v63", name="v63")
                nc.vector.tensor_tensor(v63[:], s[:], rinv[:], op=OP.mult)
                r_ = stp.tile([128, 32], f32, tag="r_", name="r_")
                nc.vector.tensor_scalar(r_[:].bitcast(i32), v63[:].bitcast(i32),
                                        1, None, op0=OP.logical_shift_right)
                nc.vector.tensor_scalar(r_[:].bitcast(i32), r_[:].bitcast(i32),
                                        -1, 0x5F3759DF, op0=OP.mult, op1=OP.add)
                nt = stp.tile([128, 32], f32, tag="nt", name="nt")
                for _ in range(2):
                    nc.vector.tensor_tensor(nt[:], v63[:], r_[:], op=OP.mult)
                    nc.vector.tensor_tensor(nt[:], nt[:], r_[:], op=OP.mult)
                    nc.vector.tensor_scalar(nt[:], nt[:], -0.5, 1.5,
                                            op0=OP.mult, op1=OP.add)
                    nc.vector.tensor_tensor(r_[:], r_[:], nt[:], op=OP.mult)
                R_ = stp.tile([128, 32], f32, tag="R_", name="R_")
                nc.vector.tensor_scalar(R_[:], r_[:], float(np.sqrt(DK - 1.0)),
                                        None, op0=OP.mult)
                u_ = stp.tile([128, 32], f32, tag="u_", name="u_")
                nc.vector.tensor_scalar(u_[:], R_[:], -EPS, 1.0,
                                        op0=OP.mult, op1=OP.add)
                g = stp.tile([128, 32], f32, tag="g", name="g")
                nc.vector.tensor_tensor(g[:], R_[:], u_[:], op=OP.mult)
                zr = stp.tile([128, 32], f32, tag="zr", name="zr")
                nc.vector.reciprocal(zr[:], Zs)
                nc.vector.tensor_tensor(w1_all[:, c0:c1], g[:], zr[:],
                                        op=OP.mult)
                gs = stp.tile([128, 8], f32, tag="gs", name="gs")
                nc.vector.tensor_reduce(
                    gs[:], g[:].rearrange("p (s c) -> p s c", c=4), axis=AX.X,
                    op=OP.add)
                nc.vector.tensor_scalar(w0_all[:, h * 8:(h + 1) * 8], gs[:],
                                        -1.0 / DK, None, op0=OP.mult)
                pw = psum_sc()
                nc.tensor.matmul(pw[:8, 0:128], w0_all[:, h * 8:(h + 1) * 8],
                                 idf_s[:], is_transpose=True, start=True,
                                 stop=True)
                w0Th = w0T0 if h == 0 else w0T1
                nc.vector.tensor_copy(w0Th[:, :], pw[:8, 0:128])
                nc.gpsimd.dma_start(
                    w0f[0:1, h * 1024:(h + 1) * 1024]
                    .rearrange("o (s f) -> o s f", s=8),
                    w0Th[:, :])

            # ---- P5 per b: heads(top of hq) = al*((sum_c e*w1)^T + w0) + b4
            def emit_p5(b):
                w1e = p5p.tile([128, 16, DK], bf16, tag="w1e")
                nc.gpsimd.tensor_copy(
                    w1e[:],
                    w1_all[:, b * 16:(b + 1) * 16].unsqueeze(-1)
                    .broadcast_to((128, 16, DK)))
                # bsc_t stored [p, ic, d, c] so the c-reduce reads stride-1
                bsc_t = p5p.tile([128, 4, DK, 4], bf16, tag="bsct")
                nc.gpsimd.tensor_tensor(
                    bsc_t[:].rearrange("p i d c -> p i c d"),
                    e_all[:, b, 0, :].rearrange("p (i c d) -> p i c d",
                                                c=4, d=DK),
                    w1e[:].rearrange("p (i c) d -> p i c d", c=4),
                    op=OP.mult)
                ball = p5p.tile([128, 4, DK], bf16, tag="ball")
                with nc.allow_low_precision("4-term c-sum tolerates bf16"):
                    nc.vector.tensor_reduce(ball[:], bsc_t[:], axis=AX.X,
                                            op=OP.add)
                pbig = psum_sc()
                for ic in range(4):
                    nc.tensor.matmul(pbig[0:64, ic * 128:(ic + 1) * 128],
                                     ball[:, ic, :], id_s[:],
                                     start=True, stop=False,
                                     skip_group_check=True)
                    slot = b * 4 + ic
                    nc.tensor.matmul(
                        pbig[0:64, ic * 128:(ic + 1) * 128], ones_f[:, 0:DK],
                        w0f[0:1, slot * 128:(slot + 1) * 128],
                        start=False, stop=True, skip_group_check=True,
                    )
                nc.vector.tensor_scalar(
                    hq[0:64, b, :], pbig[0:64, :],
                    al_s[:], b4_s[:], op0=OP.mult, op1=OP.add,
                )

            # ---- P6 per b: out = WoP^T @ [heads; qhT] (+bo) ----
            def emit_p6(b):
                for nch in range(4):
                    po = psum_su()
                    nc.tensor.matmul(po[:], WoP_s[:, nch, :], hq[:, b, :],
                                     start=True, stop=True)
                    ot = otp.tile([128, S], bf16, tag="ot")
                    if b < 2:
                        nc.vector.tensor_scalar(
                            ot[:], po[:], bo2_s[:, nch:nch + 1], None,
                            op0=OP.add)
                    else:
                        nc.scalar.activation(ot[:], po[:], AF.Identity,
                                             bias=bo2_s[:, nch:nch + 1])
                    nc.sync.dma_start(outT_d[b, nch * 128:(nch + 1) * 128, :],
                                     ot[:])

            # ---- emission schedule (engine-queue order is the priority) ----
            for kb in range(4):
                emit_kproj(kb)
            emit_qproj(0)
            # early Exp table load while E(0,0) psum fills
            nc.scalar.activation(warm_z[0:1, 0:8], warm_z[0:1, 0:8], AF.Exp)

            ex = emit_e(0, 0)
            emit_fenmu_rt(0, 0, ex, nc.gpsimd)
            emit_bvb()
            emit_vh(0)
            ex = emit_e(0, 1)
            emit_fenmu_rt(0, 1, ex, nc.vector)
            emit_qproj(1)
            ex = emit_e(0, 2)
            emit_fenmu_rt(0, 2, ex, nc.gpsimd)
            emit_vh(1)
            ex = emit_e(0, 3)
            emit_fenmu_rt(0, 3, ex, nc.gpsimd)
            emit_qproj(2)

            ex = emit_e(1, 0)
            emit_fenmu_rt(1, 0, ex, nc.gpsimd)
            emit_vh(2)
            ex = emit_e(1, 1)
            emit_fenmu_rt(1, 1, ex, nc.vector)
            emit_qproj(3)
            ex = emit_e(1, 2)
            emit_fenmu_rt(1, 2, ex, nc.gpsimd)
            emit_vh(3)
            ex = emit_e(1, 3)
            emit_fenmu_rt(1, 3, ex, nc.gpsimd)

            emit_p2(2, rt_gp=[True, False, True, True])
            emit_p3(0)
            emit_p2(3, rt_gp=[True, False, True, True])
            emit_p3(1)
            emit_stats(0)
            emit_p3(2)
            emit_p5(0)
            emit_p6(0)
            emit_p3(3)
            emit_p5(1)
            emit_p6(1)
            emit_stats(1)
            emit_p5(2)
            emit_p6(2)
            emit_p5(3)
            emit_p6(3)

    return nc


def _build():
    import concourse.bass as bass  # noqa
    import concourse.tile as tile
    from concourse import bacc, mybir

    nc = bacc.Bacc("TRN2", target_bir_lowering=False, debug=False,
                   num_devices=NCORES)
    build_program(nc, tile, mybir)
    nc.compile()
    return nc


_cached_nc = None


def make_in_maps(q, k, v, Wq, bq, Wk, bk, Wv, bv, Wo, bo, alpha, beta):
    import ml_dtypes
    bft = ml_dtypes.bfloat16

    def prelay(x):
        # [S, DM] per batch -> transposed [DM, S] -> [128, 4, S] layout
        xT = np.swapaxes(np.asarray(x, np.float32), 1, 2)  # [B, DM, S]
        return np.ascontiguousarray(
            xT.reshape(BS, 4, 128, S).transpose(0, 2, 1, 3)).astype(bft)

    def wlay(W):  # [DM, DK] -> [128, 4, DK]
        return np.ascontiguousarray(
            np.asarray(W, np.float32).reshape(4, 128, DK).transpose(1, 0, 2)
        ).astype(bft)

    qT, kT, vT = prelay(q), prelay(k), prelay(v)
    Wq, Wk, Wv, Wo = (np.asarray(x, np.float32) for x in (Wq, Wk, Wv, Wo))
    bq, bk, bv, bo = (np.asarray(x, np.float32) for x in (bq, bk, bv, bo))
    alpha, beta = np.asarray(alpha, np.float32), np.asarray(beta, np.float32)
    ident = np.eye(128, dtype=ml_dtypes.bfloat16)
    identf = np.eye(128, dtype=np.float32)
    scale = np.float32(1.0 / np.sqrt(np.float32(DK)))  # fenmu sqrt(DK) -> Wv
    in_maps = []
    for h in range(NCORES):
        sl = slice(h * DK, (h + 1) * DK)
        WoP = np.zeros((128, 4, 128), np.float32)
        for nch in range(4):
            WoP[0:64, nch, :] = Wo[sl, nch * 128:(nch + 1) * 128]
            WoP[64:128, nch, :] = 4.0 * Wo[sl, nch * 128:(nch + 1) * 128]
        in_maps.append({
            "qT": qT, "kT": kT, "vT": vT,
            "Wq": wlay(Wq[:, sl]),
            "Wk": wlay(Wk[:, sl]),
            "Wv": wlay(Wv[:, sl] * scale),
            "bqc": np.ascontiguousarray(bq[sl])[:, None].astype(np.float32),
            "bkc": np.ascontiguousarray(bk[sl])[:, None].astype(np.float32),
            "bv": np.ascontiguousarray(bv[sl] * scale)[None, :].astype(bft),
            "WoP": WoP.astype(bft),
            "bo2": np.ascontiguousarray(
                (bo if h == 0 else np.zeros_like(bo)).reshape(4, 128).T
            ).astype(np.float32),
            "alpha": np.ascontiguousarray(alpha)[:, None],
            "beta4": np.ascontiguousarray(4.0 * beta)[:, None],
            "ident": ident, "identf": identf,
        })
    return in_maps


def assemble(results):
    out = np.zeros((BS, S, DM), np.float32)
    for r in results:
        out += np.swapaxes(np.asarray(r["outT"], np.float32), 1, 2)
    return out


def kernel(**inputs) -> np.ndarray:
    global _cached_nc
    from concourse.bass_utils import run_bass_kernel_spmd

    if _cached_nc is None:
        _cached_nc = _build()
    in_maps = make_in_maps(**inputs)
    res = run_bass_kernel_spmd(_cached_nc, in_maps, list(range(NCORES)))
    return assemble(res.results)


# revision 10
# speedup vs baseline: 1.0922x; 1.0922x over previous
"""Trainium2 Bass kernel for nn_MultiHeadCrossAttention (BS=4, S=512, DM=512, H=8).

Sharding: one attention head per NeuronCore (8 heads / 8 cores). Each core
receives the full (transposed) q/k/v plus its head's weight slices, computes
its head end-to-end including the rank-64 slice of the output projection, and
the host sums the 8 partial outputs.

v3 structure (drives the ACT exp stream at full rate; PE never idles long):
  - All PSUM flows through ONE tag [128, 4, 512] x 2 bufs (8 banks): E scores,
    fenmu (reuses slot 0 of the consumed tile post-exp), P3 scores, vh, P5,
    P6 -- the 2-deep rotation double-buffers the E-unit pipeline so the
    [128,4,512] exp activation (~2us) streams back to back.
  - P2 unit (b, jc): 4 E matmuls (K=64) -> one exp act -> 4 identity-matmul
    fenmu accumulation into slot 0 -> reciprocal_approx_fast -> rt (GPSIMD).
  - P6 packed to K=128: out = [Wo; 4*Wo]^T @ [heads; qhT].
  - e2 = e*e on DVE; e stored bf16; rt-mults on GPSIMD; everything else DVE.
  - Input DMAs lead the sync queue (kT first); weights ride the scalar
    (ACT) HWDGE queue in parallel; PE warmup matmuls cover the ramp (HAM).
"""

import numpy as np

BS, S, DM, H, DK = 4, 512, 512, 8, 64
EPS = 1e-6
NCORES = 8


def build_program(nc, tile, mybir, bo_zero):
    f32 = mybir.dt.float32
    bf16 = mybir.dt.bfloat16
    i32 = mybir.dt.int32
    AF = mybir.ActivationFunctionType
    OP = mybir.AluOpType
    AX = mybir.AxisListType

    # ---- DRAM I/O ----
    qT_d = nc.dram_tensor("qT", [BS, 128, 4, S], bf16, kind="ExternalInput")
    kT_d = nc.dram_tensor("kT", [BS, 128, 4, S], bf16, kind="ExternalInput")
    vT_d = nc.dram_tensor("vT", [BS, 128, 4, S], bf16, kind="ExternalInput")
    Wq_d = nc.dram_tensor("Wq", [128, 4, DK], bf16, kind="ExternalInput")
    Wk_d = nc.dram_tensor("Wk", [128, 4, DK], bf16, kind="ExternalInput")
    Wv_d = nc.dram_tensor("Wv", [128, 4, DK], bf16, kind="ExternalInput")
    bqc_d = nc.dram_tensor("bqc", [DK, 1], f32, kind="ExternalInput")
    bkc_d = nc.dram_tensor("bkc", [DK, 1], f32, kind="ExternalInput")
    bv_d = nc.dram_tensor("bv", [1, DK], bf16, kind="ExternalInput")
    WoP_d = nc.dram_tensor("WoP", [128, 4, 128], bf16, kind="ExternalInput")
    bo2_d = nc.dram_tensor("bo2", [128, 4], f32, kind="ExternalInput")
    al_d = nc.dram_tensor("alpha", [DK, 1], f32, kind="ExternalInput")
    b4_d = nc.dram_tensor("beta4", [DK, 1], f32, kind="ExternalInput")
    id_d = nc.dram_tensor("ident", [128, 128], bf16, kind="ExternalInput")
    idf_d = nc.dram_tensor("identf", [128, 128], f32, kind="ExternalInput")
    outT_d = nc.dram_tensor("outT", [BS, DM, S], bf16, kind="ExternalOutput")

    with tile.TileContext(nc) as tc:
        with (
            tc.tile_pool(name="persist", bufs=1) as pp,
            tc.tile_pool(name="consts", bufs=1) as cp,
            tc.tile_pool(name="kin", bufs=1) as kip,
            tc.tile_pool(name="vin", bufs=1) as vip,
            tc.tile_pool(name="qin", bufs=2) as qip,
            tc.tile_pool(name="exw", bufs=3) as exp_pool,
            tc.tile_pool(name="wrw", bufs=3) as wrp,
            tc.tile_pool(name="p5w", bufs=2) as p5p,
            tc.tile_pool(name="otw", bufs=2) as otp,
            tc.tile_pool(name="stats", bufs=2) as stp,
            tc.tile_pool(name="psum", bufs=1, space="PSUM") as psp,
        ):
            # ---- persistent SBUF ----
            qhT = pp.tile([DK, BS, S], bf16, tag="qhT")
            khT = pp.tile([DK, BS, S], bf16, tag="khT")
            # hq: top = heads (P5 output), bottom = qhT    (P6 rhs, K=128)
            hq = pp.tile([128, BS, S], bf16, tag="hq")
            vh_all = pp.tile([128, 4, BS, DK], bf16, tag="vh")   # [j, jc, c, d]
            rt_all = pp.tile([128, BS, 4, S], bf16, tag="rt")    # [j, b, jc, i]
            # e_all: [i, b, u, (ic c d)]  u=0: e=exp(s), u=1: e2=e*e
            e_all = pp.tile([128, BS, 2, 1024], bf16, tag="e")
            Z_all = pp.tile([128, 64], f32, tag="Z")   # cols = b*16 + ic*4 + c
            Q_all = pp.tile([128, 64], f32, tag="Q")
            w1_all = pp.tile([128, 64], f32, tag="w1")
            w0_all = pp.tile([128, 16], f32, tag="w0")  # cols = b*4 + ic
            w0T0 = pp.tile([8, 128], f32, tag="w0T0")
            w0T1 = pp.tile([8, 128], f32, tag="w0T1")
            w0f = pp.tile([1, 16 * 128], f32, tag="w0f")
            bvb = pp.tile([128, DK], bf16, tag="bvb")

            Wq_s = cp.tile([128, 4, DK], bf16, tag="Wq")
            Wk_s = cp.tile([128, 4, DK], bf16, tag="Wk")
            Wv_s = cp.tile([128, 4, DK], bf16, tag="Wv")
            WoP_s = cp.tile([128, 4, 128], bf16, tag="WoP")
            bo2_s = cp.tile([128, 4], f32, tag="bo2")
            bqc_s = cp.tile([DK, 1], f32, tag="bqc")
            bkc_s = cp.tile([DK, 1], f32, tag="bkc")
            bv_s = cp.tile([1, DK], bf16, tag="bv")
            al_s = cp.tile([DK, 1], f32, tag="al")
            b4_s = cp.tile([DK, 1], f32, tag="b4")
            id_s = cp.tile([128, 128], bf16, tag="id")
            idf_s = cp.tile([128, 128], f32, tag="idf")
            ones_b = cp.tile([1, 128], bf16, tag="ones_b")
            ones_f = cp.tile([1, 128], f32, tag="ones_f")
            warm_z = cp.tile([128, S], bf16, tag="warm_z")

            # ---- DMA: big inputs lead the sync queue ----
            for b in range(BS):
                kt = kip.tile([128, 4, S], bf16, tag=f"kt{b}")
                nc.sync.dma_start(kt[:], kT_d[b])
                ktiles = ktiles + [kt] if b else [kt]
            qtiles, vtiles = [None] * BS, [None] * BS
            for b in range(BS):
                qt = qip.tile([128, 4, S], bf16, tag="qt", name=f"qt{b}")
                nc.sync.dma_start(qt[:], qT_d[b])
                qtiles[b] = qt
                vt = vip.tile([128, 4, S], bf16, tag=f"vt{b}")
                nc.sync.dma_start(vt[:], vT_d[b])
                vtiles[b] = vt
            # small weights ride the scalar (ACT) HWDGE queue in parallel
            nc.scalar.dma_start(id_s[:], id_d[:])
            nc.scalar.dma_start(Wk_s[:], Wk_d[:])
            nc.scalar.dma_start(Wq_s[:], Wq_d[:])
            nc.scalar.dma_start(Wv_s[:], Wv_d[:])
            nc.scalar.dma_start(bqc_s[:], bqc_d[:])
            nc.scalar.dma_start(bkc_s[:], bkc_d[:])
            nc.scalar.dma_start(bv_s[:], bv_d[:])
            nc.scalar.dma_start(WoP_s[:], WoP_d[:])
            nc.scalar.dma_start(bo2_s[:], bo2_d[:])
            nc.scalar.dma_start(al_s[:], al_d[:])
            nc.scalar.dma_start(b4_s[:], b4_d[:])
            nc.scalar.dma_start(idf_s[:], idf_d[:])

            nc.vector.memset(ones_b[:], 1.0)
            nc.vector.memset(ones_f[:], 1.0)
            nc.vector.memset(warm_z[:], 0.0)

            # ---- single PSUM rotation: [128, 4, 512] x 2 bufs = 8 banks ----
            def psum_t(name="ppe"):
                return psp.tile([128, 4, S], f32, tag="pe", bufs=2, name=name)

            # ---- PE warmup (HAM clock-gate) during kT0 transfer ----
            wps = psum_t("warm")
            for w in range(8):
                nc.tensor.matmul(wps[:, w % 4, :], id_s[:], warm_z[:],
                                 start=True, stop=True)

            # ---- P1 projections (psum partitions 0-63, slot 0) ----
            def emit_proj(W_s, b_c, srct, dst_ap):
                ps = psum_t("pproj")
                for mc in range(4):
                    nc.tensor.matmul(ps[0:DK, 0, :], W_s[:, mc, :],
                                     srct[:, mc, :],
                                     start=(mc == 0), stop=(mc == 3))
                nc.scalar.activation(dst_ap, ps[0:DK, 0, :], AF.Identity,
                                     bias=b_c[:])

            # ---- bvb: broadcast bv across partitions via K=1 matmul ----
            def emit_bvb():
                pb = psum_t("pbv")
                nc.tensor.matmul(pb[:, 0, 0:DK], ones_b[:, :], bv_s[:],
                                 start=True, stop=True)
                nc.vector.tensor_copy(bvb[:], pb[:, 0, 0:DK])

            # ---- vh per c: vh_all[j, jc, c, d] = (vt @ Wv) + bv ----
            def emit_vh(c):
                vt = vtiles[c]
                pv = psum_t("pvh")
                for jc in range(4):
                    for mc in range(4):
                        nc.tensor.matmul(
                            pv[:, 0, jc * DK:(jc + 1) * DK],
                            vt[:, mc, jc * 128:(jc + 1) * 128],
                            Wv_s[:, mc, :],
                            start=(mc == 0), stop=(mc == 3),
                        )
                nc.vector.tensor_tensor(
                    vh_all[:, :, c, :],
                    pv[:, 0, 0:4 * DK].rearrange("p (jc d) -> p jc d", d=DK),
                    bvb[:].unsqueeze(1).broadcast_to((128, 4, DK)),
                    op=OP.add)

            # ---- P2 unit (b, jc) ----
            def emit_p2_unit(b, jc):
                ph = psum_t("ppe")
                js = slice(jc * 128, (jc + 1) * 128)
                for kb in range(4):
                    nc.tensor.matmul(ph[:, kb, :], khT[:, kb, js],
                                     qhT[:, b, :], start=True, stop=True)
                ex = exp_pool.tile([128, 4, S], bf16, tag="ex")
                nc.scalar.activation(ex[:], ph[:], AF.Exp)
                # fenmu into slot 0 (freed by the exp read), then 1/fenmu, rt
                for kb in range(4):
                    nc.tensor.matmul(ph[:, 0, :], id_s[:], ex[:, kb, :],
                                     start=(kb == 0), stop=(kb == 3))
                wr = wrp.tile([128, S], f32, tag="wr")
                nc.vector.reciprocal_approx_fast(wr[:], ph[:, 0, :])
                nc.gpsimd.tensor_tensor(rt_all[:, b, jc, :], ex[:, b, :],
                                        wr[:], op=OP.mult)

            # ---- P3 per b: scores + e/e2 + Z/Q ----
            def emit_p3(b):
                ps3 = psum_t("p3")
                for ic in range(4):
                    for jc in range(4):
                        nc.tensor.matmul(
                            ps3[:, ic // 2, (ic % 2) * 256:(ic % 2 + 1) * 256],
                            rt_all[:, b, jc, ic * 128:(ic + 1) * 128],
                            vh_all[:, jc].rearrange("p c d -> p (c d)"),
                            start=(jc == 0), stop=(jc == 3),
                        )
                nc.scalar.activation(
                    e_all[:, b, 0, :].rearrange("p (a i) -> p a i", a=2),
                    ps3[:, 0:2, :], AF.Exp)
                nc.vector.tensor_tensor(e_all[:, b, 1, :], e_all[:, b, 0, :],
                                        e_all[:, b, 0, :], op=OP.mult)
                nc.vector.tensor_reduce(
                    Z_all[:, b * 16:(b + 1) * 16],
                    e_all[:, b, 0, :].rearrange("p (g d) -> p g d", d=DK),
                    axis=AX.X, op=OP.add)
                nc.vector.tensor_reduce(
                    Q_all[:, b * 16:(b + 1) * 16],
                    e_all[:, b, 1, :].rearrange("p (g d) -> p g d", d=DK),
                    axis=AX.X, op=OP.add)

            # ---- P4: stats per b-pair h (w1, w0) — DVE bit-trick rsqrt ----
            def emit_stats(h):
                c0, c1 = h * 32, (h + 1) * 32
                Zs, Qs = Z_all[:, c0:c1], Q_all[:, c0:c1]
                t = stp.tile([128, 32], f32, tag="t", name="t")
                nc.vector.tensor_tensor(t[:], Zs, Zs, op=OP.mult)
                s = stp.tile([128, 32], f32, tag="s", name="s")
                nc.vector.scalar_tensor_tensor(
                    s[:], t[:], -1.0 / DK, Qs, op0=OP.mult, op1=OP.add)
                rinv = stp.tile([128, 32], f32, tag="rinv", name="rinv")
                nc.vector.reciprocal(rinv[:], t[:])
                v63 = stp.tile([128, 32], f32, tag="v63", name="v63")
                nc.vector.tensor_tensor(v63[:], s[:], rinv[:], op=OP.mult)
                r_ = stp.tile([128, 32], f32, tag="r_", name="r_")
                nc.vector.tensor_scalar(r_[:].bitcast(i32), v63[:].bitcast(i32),
                                        1, None, op0=OP.logical_shift_right)
                nc.vector.tensor_scalar(r_[:].bitcast(i32), r_[:].bitcast(i32),
                                        -1, 0x5F3759DF, op0=OP.mult, op1=OP.add)
                nt = stp.tile([128, 32], f32, tag="nt", name="nt")
                for _ in range(2):
                    nc.vector.tensor_tensor(nt[:], v63[:], r_[:], op=OP.mult)
                    nc.vector.tensor_tensor(nt[:], nt[:], r_[:], op=OP.mult)
                    nc.vector.tensor_scalar(nt[:], nt[:], -0.5, 1.5,
                                            op0=OP.mult, op1=OP.add)
                    nc.vector.tensor_tensor(r_[:], r_[:], nt[:], op=OP.mult)
                R_ = stp.tile([128, 32], f32, tag="R_", name="R_")
                nc.vector.tensor_scalar(R_[:], r_[:], float(np.sqrt(DK - 1.0)),
                                        None, op0=OP.mult)
                u_ = stp.tile([128, 32], f32, tag="u_", name="u_")
                nc.vector.tensor_scalar(u_[:], R_[:], -EPS, 1.0,
                                        op0=OP.mult, op1=OP.add)
                g = stp.tile([128, 32], f32, tag="g", name="g")
                nc.vector.tensor_tensor(g[:], R_[:], u_[:], op=OP.mult)
                zr = stp.tile([128, 32], f32, tag="zr", name="zr")
                nc.vector.reciprocal(zr[:], Zs)
                nc.vector.tensor_tensor(w1_all[:, c0:c1], g[:], zr[:],
                                        op=OP.mult)
                gs = stp.tile([128, 8], f32, tag="gs", name="gs")
                nc.vector.tensor_reduce(
                    gs[:], g[:].rearrange("p (s c) -> p s c", c=4), axis=AX.X,
                    op=OP.add)
                nc.vector.tensor_scalar(w0_all[:, h * 8:(h + 1) * 8], gs[:],
                                        -1.0 / DK, None, op0=OP.mult)
                pw = psum_t("pw")
                nc.tensor.matmul(pw[:8, 0, 0:128],
                                 w0_all[:, h * 8:(h + 1) * 8],
                                 idf_s[:], is_transpose=True, start=True,
                                 stop=True)
                w0Th = w0T0 if h == 0 else w0T1
                nc.vector.tensor_copy(w0Th[:, :], pw[:8, 0, 0:128])
                nc.sync.dma_start(
                    w0f[0:1, h * 1024:(h + 1) * 1024]
                    .rearrange("o (s f) -> o s f", s=8),
                    w0Th[:, :])

            # ---- P5 per b (DVE prep + PE transpose/rank-1) ----
            def emit_p5_prep(b):
                w1e = p5p.tile([128, 16, DK], bf16, tag="w1e")
                nc.vector.tensor_copy(
                    w1e[:],
                    w1_all[:, b * 16:(b + 1) * 16].unsqueeze(-1)
                    .broadcast_to((128, 16, DK)))
                # bsc_t stored [p, ic, d, c] so the c-reduce reads stride-1
                bsc_t = p5p.tile([128, 4, DK, 4], bf16, tag="bsct")
                nc.vector.tensor_tensor(
                    bsc_t[:].rearrange("p i d c -> p i c d"),
                    e_all[:, b, 0, :].rearrange("p (i c d) -> p i c d",
                                                c=4, d=DK),
                    w1e[:].rearrange("p (i c) d -> p i c d", c=4),
                    op=OP.mult)
                ball = p5p.tile([128, 4, DK], bf16, tag="ball")
                with nc.allow_low_precision("4-term c-sum tolerates bf16"):
                    nc.vector.tensor_reduce(ball[:], bsc_t[:], axis=AX.X,
                                            op=OP.add)
                return ball

            def emit_p5(b, ball):
                pbig = psum_t("p5")
                for ic in range(4):
                    nc.tensor.matmul(pbig[0:64, 0,
                                          ic * 128:(ic + 1) * 128],
                                     ball[:, ic, :], id_s[:],
                                     start=True, stop=False,
                                     skip_group_check=True)
                    slot = b * 4 + ic
                    nc.tensor.matmul(
                        pbig[0:64, 0, ic * 128:(ic + 1) * 128],
                        ones_f[:, 0:DK],
                        w0f[0:1, slot * 128:(slot + 1) * 128],
                        start=False, stop=True, skip_group_check=True,
                    )
                nc.vector.tensor_scalar(
                    hq[0:64, b, :], pbig[0:64, 0, :],
                    al_s[:], b4_s[:], op0=OP.mult, op1=OP.add,
                )

            # ---- P6 per b: out = WoP^T @ [heads; qhT] (+bo) ----
            def emit_p6(b):
                po = psum_t("p6")
                for nch in range(4):
                    nc.tensor.matmul(po[:, nch, :], WoP_s[:, nch, :],
                                     hq[:, b, :], start=True, stop=True)
                ot = otp.tile([128, 4, S], bf16, tag="ot")
                if bo_zero:
                    if b < 2:
                        nc.vector.tensor_copy(ot[:], po[:])
                    else:
                        nc.scalar.activation(ot[:], po[:], AF.Identity)
                else:
                    eng = nc.vector if b < 2 else None
                    for nch in range(4):
                        if eng is not None:
                            eng.tensor_scalar(ot[:, nch, :], po[:, nch, :],
                                              bo2_s[:, nch:nch + 1], None,
                                              op0=OP.add)
                        else:
                            nc.scalar.activation(ot[:, nch, :], po[:, nch, :],
                                                 AF.Identity,
                                                 bias=bo2_s[:, nch:nch + 1])
                nc.sync.dma_start(
                    outT_d[b].rearrange("(n p) i -> p n i", p=128), ot[:])

            # ---- emission schedule (engine-queue order is the priority) ----
            for kb in range(4):
                emit_proj(Wk_s, bkc_s, ktiles[kb], khT[:, kb, :])
                if kb == 1:
                    # Exp table load in the ACT gap between k-proj copies
                    nc.scalar.activation(warm_z[0:1, 0:8], warm_z[0:1, 0:8],
                                         AF.Exp)
            emit_proj(Wq_s, bqc_s, qtiles[0], qhT[:, 0, :])
            nc.scalar.dma_start(hq[64:128, 0, :], qhT[:, 0, :])

            emit_p2_unit(0, 0)
            emit_bvb()
            emit_p2_unit(0, 1)
            emit_vh(0)
            emit_p2_unit(0, 2)
            emit_proj(Wq_s, bqc_s, qtiles[1], qhT[:, 1, :])
            nc.scalar.dma_start(hq[64:128, 1, :], qhT[:, 1, :])
            emit_p2_unit(0, 3)
            emit_vh(1)
            emit_p2_unit(1, 0)
            emit_proj(Wq_s, bqc_s, qtiles[2], qhT[:, 2, :])
            nc.scalar.dma_start(hq[64:128, 2, :], qhT[:, 2, :])
            emit_p2_unit(1, 1)
            emit_vh(2)
            emit_p2_unit(1, 2)
            emit_proj(Wq_s, bqc_s, qtiles[3], qhT[:, 3, :])
            nc.scalar.dma_start(hq[64:128, 3, :], qhT[:, 3, :])
            emit_p2_unit(1, 3)
            emit_vh(3)

            emit_p2_unit(2, 0)
            emit_p2_unit(2, 1)
            emit_p2_unit(2, 2)
            emit_p2_unit(2, 3)
            emit_p3(0)
            emit_p2_unit(3, 0)
            emit_p2_unit(3, 1)
            emit_p3(1)
            emit_p2_unit(3, 2)
            emit_stats(0)
            emit_p2_unit(3, 3)
            emit_p3(2)
            ball0 = emit_p5_prep(0)
            emit_p3(3)
            emit_p5(0, ball0)
            emit_p6(0)
            ball1 = emit_p5_prep(1)
            emit_p5(1, ball1)
            emit_p6(1)
            emit_stats(1)
            ball2 = emit_p5_prep(2)
            emit_p5(2, ball2)
            emit_p6(2)
            ball3 = emit_p5_prep(3)
            emit_p5(3, ball3)
            emit_p6(3)

    return nc


def _build(bo_zero):
    import concourse.bass as bass  # noqa
    import concourse.tile as tile
    from concourse import bacc, mybir

    nc = bacc.Bacc("TRN2", target_bir_lowering=False, debug=False,
                   num_devices=NCORES)
    build_program(nc, tile, mybir, bo_zero)
    nc.compile()
    return nc


_cached_nc = None
_cached_bo_zero = None


def make_in_maps(q, k, v, Wq, bq, Wk, bk, Wv, bv, Wo, bo, alpha, beta):
    import ml_dtypes
    bft = ml_dtypes.bfloat16

    def prelay(x):
        # [S, DM] per batch -> transposed [DM, S] -> [128, 4, S] layout
        xT = np.swapaxes(np.asarray(x, np.float32), 1, 2)  # [B, DM, S]
        return np.ascontiguousarray(
            xT.reshape(BS, 4, 128, S).transpose(0, 2, 1, 3)).astype(bft)

    def wlay(W):  # [DM, DK] -> [128, 4, DK]
        return np.ascontiguousarray(
            np.asarray(W, np.float32).reshape(4, 128, DK).transpose(1, 0, 2)
        ).astype(bft)

    qT, kT, vT = prelay(q), prelay(k), prelay(v)
    Wq, Wk, Wv, Wo = (np.asarray(x, np.float32) for x in (Wq, Wk, Wv, Wo))
    bq, bk, bv, bo = (np.asarray(x, np.float32) for x in (bq, bk, bv, bo))
    alpha, beta = np.asarray(alpha, np.float32), np.asarray(beta, np.float32)
    ident = np.eye(128, dtype=ml_dtypes.bfloat16)
    identf = np.eye(128, dtype=np.float32)
    scale = np.float32(1.0 / np.sqrt(np.float32(DK)))  # fenmu sqrt(DK) -> Wv
    in_maps = []
    for h in range(NCORES):
        sl = slice(h * DK, (h + 1) * DK)
        WoP = np.zeros((128, 4, 128), np.float32)
        for nch in range(4):
            WoP[0:64, nch, :] = Wo[sl, nch * 128:(nch + 1) * 128]
            WoP[64:128, nch, :] = 4.0 * Wo[sl, nch * 128:(nch + 1) * 128]
        in_maps.append({
            "qT": qT, "kT": kT, "vT": vT,
            "Wq": wlay(Wq[:, sl]),
            "Wk": wlay(Wk[:, sl]),
            "Wv": wlay(Wv[:, sl] * scale),
            "bqc": np.ascontiguousarray(bq[sl])[:, None].astype(np.float32),
            "bkc": np.ascontiguousarray(bk[sl])[:, None].astype(np.float32),
            "bv": np.ascontiguousarray(bv[sl] * scale)[None, :].astype(bft),
            "WoP": WoP.astype(bft),
            "bo2": np.ascontiguousarray(
                (bo if h == 0 else np.zeros_like(bo)).reshape(4, 128).T
            ).astype(np.float32),
            "alpha": np.ascontiguousarray(alpha)[:, None],
            "beta4": np.ascontiguousarray(4.0 * beta)[:, None],
            "ident": ident, "identf": identf,
        })
    return in_maps


def assemble(results):
    out = np.zeros((BS, S, DM), np.float32)
    for r in results:
        out += np.swapaxes(np.asarray(r["outT"], np.float32), 1, 2)
    return out


def kernel(**inputs) -> np.ndarray:
    global _cached_nc, _cached_bo_zero
    from concourse.bass_utils import run_bass_kernel_spmd

    bo_zero = bool(np.all(np.asarray(inputs["bo"]) == 0.0))
    if _cached_nc is None or _cached_bo_zero != bo_zero:
        _cached_nc = _build(bo_zero)
        _cached_bo_zero = bo_zero
    in_maps = make_in_maps(**inputs)
    res = run_bass_kernel_spmd(_cached_nc, in_maps, list(range(NCORES)))
    return assemble(res.results)


# revision 18
# speedup vs baseline: 1.0929x; 1.0007x over previous
"""Trainium2 Bass kernel for nn_MultiHeadCrossAttention (BS=4, S=512, DM=512, H=8).

Sharding: one attention head per NeuronCore (8 heads / 8 cores). Each core
receives the full (transposed) q/k/v plus its head's weight slices, computes
its head end-to-end including the rank-64 slice of the output projection, and
the host sums the 8 partial outputs.

v4 structure:
  - Single PSUM rotation [128,4,512] x 2 bufs (8 banks) for everything.
  - P2 software-pipelined: E matmuls of unit n+1 are emitted BEFORE the
    fenmu/wrec/rt tail of unit n, so the ~2us exp activation streams
    back-to-back (the fenmu identity-matmuls depend on the exp output and
    otherwise serialize the PE queue).
  - Exp is the only ACT function inside the P2 window (q-proj copies for
    b>=1 go через DVE) -- avoids ~1.3us activation-table reloads.
  - P6 packed to K=128 (out = [Wo; 4Wo]^T @ [heads; qhT]); output copies on
    ACT (idle in the tail).  rt-mults on GPSIMD; tail elementwise split
    DVE/GPSIMD.  PE warmup matmuls at t0 for the HAM clock gate.
"""

import numpy as np

BS, S, DM, H, DK = 4, 512, 512, 8, 64
EPS = 1e-6
NCORES = 8


def build_program(nc, tile, mybir, bo_zero):
    f32 = mybir.dt.float32
    bf16 = mybir.dt.bfloat16
    i32 = mybir.dt.int32
    AF = mybir.ActivationFunctionType
    OP = mybir.AluOpType
    AX = mybir.AxisListType

    # ---- DRAM I/O ----
    qT_d = nc.dram_tensor("qT", [BS, 128, 4, S], bf16, kind="ExternalInput")
    kT_d = nc.dram_tensor("kT", [BS, 128, 4, S], bf16, kind="ExternalInput")
    vT_d = nc.dram_tensor("vT", [BS, 128, 4, S], bf16, kind="ExternalInput")
    Wq_d = nc.dram_tensor("Wq", [128, 4, DK], bf16, kind="ExternalInput")
    Wk_d = nc.dram_tensor("Wk", [128, 4, DK], bf16, kind="ExternalInput")
    Wv_d = nc.dram_tensor("Wv", [128, 4, DK], bf16, kind="ExternalInput")
    bqc_d = nc.dram_tensor("bqc", [DK, 1], f32, kind="ExternalInput")
    bkc_d = nc.dram_tensor("bkc", [DK, 1], f32, kind="ExternalInput")
    bv_d = nc.dram_tensor("bv", [1, DK], bf16, kind="ExternalInput")
    WoP_d = nc.dram_tensor("WoP", [128, 4, 128], bf16, kind="ExternalInput")
    bo2_d = nc.dram_tensor("bo2", [128, 4], f32, kind="ExternalInput")
    al_d = nc.dram_tensor("alpha", [DK, 1], f32, kind="ExternalInput")
    b4_d = nc.dram_tensor("beta4", [DK, 1], f32, kind="ExternalInput")
    id_d = nc.dram_tensor("ident", [128, 128], bf16, kind="ExternalInput")
    idf_d = nc.dram_tensor("identf", [128, 128], f32, kind="ExternalInput")
    outT_d = nc.dram_tensor("outT", [BS, DM, S], bf16, kind="ExternalOutput")

    with tile.TileContext(nc) as tc:
        with (
            tc.tile_pool(name="persist", bufs=1) as pp,
            tc.tile_pool(name="consts", bufs=1) as cp,
            tc.tile_pool(name="kin", bufs=1) as kip,
            tc.tile_pool(name="vin", bufs=1) as vip,
            tc.tile_pool(name="qin", bufs=2) as qip,
            tc.tile_pool(name="exw", bufs=3) as exp_pool,
            tc.tile_pool(name="wrw", bufs=3) as wrp,
            tc.tile_pool(name="p5w", bufs=2) as p5p,
            tc.tile_pool(name="otw", bufs=2) as otp,
            tc.tile_pool(name="stats", bufs=2) as stp,
            tc.tile_pool(name="psum", bufs=1, space="PSUM") as psp,
        ):
            # ---- persistent SBUF ----
            qhT = pp.tile([DK, BS, S], bf16, tag="qhT")
            khT = pp.tile([DK, BS, S], bf16, tag="khT")
            # hq: top = heads (P5 output), bottom = qhT    (P6 rhs, K=128)
            hq = pp.tile([128, BS, S], bf16, tag="hq")
            vh_all = pp.tile([128, 4, BS, DK], bf16, tag="vh")   # [j, jc, c, d]
            rt_all = pp.tile([128, BS, 4, S], bf16, tag="rt")    # [j, b, jc, i]
            # e_all: [i, b, u, (ic c d)]  u=0: e=exp(s), u=1: e2=e*e
            e_all = pp.tile([128, BS, 2, 1024], bf16, tag="e")
            Z_all = pp.tile([128, 64], f32, tag="Z")   # cols = b*16 + ic*4 + c
            Q_all = pp.tile([128, 64], f32, tag="Q")
            w1_all = pp.tile([128, 64], f32, tag="w1")
            w0_all = pp.tile([128, 16], f32, tag="w0")  # cols = b*4 + ic
            w0T0 = pp.tile([8, 128], f32, tag="w0T0")
            w0T1 = pp.tile([8, 128], f32, tag="w0T1")
            w0f = pp.tile([1, 16 * 128], f32, tag="w0f")
            bvb = pp.tile([128, DK], bf16, tag="bvb")

            Wq_s = cp.tile([128, 4, DK], bf16, tag="Wq")
            Wk_s = cp.tile([128, 4, DK], bf16, tag="Wk")
            Wv_s = cp.tile([128, 4, DK], bf16, tag="Wv")
            WoP_s = cp.tile([128, 4, 128], bf16, tag="WoP")
            bo2_s = cp.tile([128, 4], f32, tag="bo2")
            bqc_s = cp.tile([DK, 1], f32, tag="bqc")
            bkc_s = cp.tile([DK, 1], f32, tag="bkc")
            bv_s = cp.tile([1, DK], bf16, tag="bv")
            al_s = cp.tile([DK, 1], f32, tag="al")
            b4_s = cp.tile([DK, 1], f32, tag="b4")
            id_s = cp.tile([128, 128], bf16, tag="id")
            idf_s = cp.tile([128, 128], f32, tag="idf")
            ones_b = cp.tile([1, 128], bf16, tag="ones_b")
            ones_f = cp.tile([1, 128], f32, tag="ones_f")
            warm_z = cp.tile([128, S], bf16, tag="warm_z")

            # ---- DMA: big inputs lead the sync queue ----
            ktiles, qtiles, vtiles = [], [None] * BS, [None] * BS
            for b in range(BS):
                kt = kip.tile([128, 4, S], bf16, tag=f"kt{b}")
                nc.sync.dma_start(kt[:], kT_d[b])
                ktiles.append(kt)
            for b in range(BS):
                qt = qip.tile([128, 4, S], bf16, tag="qt", name=f"qt{b}")
                nc.sync.dma_start(qt[:], qT_d[b])
                qtiles[b] = qt
                vt = vip.tile([128, 4, S], bf16, tag=f"vt{b}")
                nc.sync.dma_start(vt[:], vT_d[b])
                vtiles[b] = vt
            # small weights ride the scalar (ACT) HWDGE queue in parallel
            nc.scalar.dma_start(id_s[:], id_d[:])
            nc.scalar.dma_start(Wk_s[:], Wk_d[:])
            nc.scalar.dma_start(Wq_s[:], Wq_d[:])
            nc.scalar.dma_start(Wv_s[:], Wv_d[:])
            nc.scalar.dma_start(bqc_s[:], bqc_d[:])
            nc.scalar.dma_start(bkc_s[:], bkc_d[:])
            nc.scalar.dma_start(bv_s[:], bv_d[:])
            nc.scalar.dma_start(WoP_s[:], WoP_d[:])
            nc.scalar.dma_start(bo2_s[:], bo2_d[:])
            nc.scalar.dma_start(al_s[:], al_d[:])
            nc.scalar.dma_start(b4_s[:], b4_d[:])
            nc.scalar.dma_start(idf_s[:], idf_d[:])

            nc.vector.memset(ones_b[:], 1.0)
            nc.vector.memset(ones_f[:], 1.0)
            nc.vector.memset(warm_z[:], 0.0)

            # ---- single PSUM rotation: [128, 4, 512] x 2 bufs = 8 banks ----
            def psum_t(name="ppe"):
                return psp.tile([128, 4, S], f32, tag="pe", bufs=2, name=name)

            # ---- PE warmup (HAM clock-gate) during kT0 transfer ----
            wps = psum_t("warm")
            for w in range(8):
                nc.tensor.matmul(wps[:, w % 4, :], id_s[:], warm_z[:],
                                 start=True, stop=True)

            # ---- P1 projections (slot 0 own tile, or slot 3 borrowed) ----
            def emit_proj(W_s, b_c, srct, dst_ap, on_act, ph=None):
                slot = 3 if ph is not None else 0
                ps = ph if ph is not None else psum_t("pproj")
                for mc in range(4):
                    nc.tensor.matmul(ps[0:DK, slot, :], W_s[:, mc, :],
                                     srct[:, mc, :],
                                     start=(mc == 0), stop=(mc == 3))
                if on_act:
                    nc.scalar.activation(dst_ap, ps[0:DK, slot, :],
                                         AF.Identity, bias=b_c[:])
                else:
                    nc.vector.tensor_scalar(dst_ap, ps[0:DK, slot, :], b_c[:],
                                            None, op0=OP.add)

            def emit_qproj(b, ph=None):
                emit_proj(Wq_s, bqc_s, qtiles[b], qhT[:, b, :], b == 0, ph)
                (nc.scalar if b == 0 else nc.sync).dma_start(
                    hq[64:128, b, :], qhT[:, b, :])

            # ---- bvb: broadcast bv across partitions via K=1 matmul ----
            def emit_bvb():
                pb = psum_t("pbv")
                nc.tensor.matmul(pb[:, 0, 0:DK], ones_b[:, :], bv_s[:],
                                 start=True, stop=True)
                nc.vector.tensor_copy(bvb[:], pb[:, 0, 0:DK])

            # ---- vh per c (borrows slot 1 of a consumed P2 tile) ----
            def emit_vh(c, ph):
                vt = vtiles[c]
                for jc in range(4):
                    for mc in range(4):
                        nc.tensor.matmul(
                            ph[:, 1, jc * DK:(jc + 1) * DK],
                            vt[:, mc, jc * 128:(jc + 1) * 128],
                            Wv_s[:, mc, :],
                            start=(mc == 0), stop=(mc == 3),
                        )
                nc.vector.tensor_tensor(
                    vh_all[:, :, c, :],
                    ph[:, 1, 0:4 * DK].rearrange("p (jc d) -> p jc d", d=DK),
                    bvb[:].unsqueeze(1).broadcast_to((128, 4, DK)),
                    op=OP.add)

            # ---- P2 unit (b, jc), software-pipelined halves ----
            def emit_e_part(b, jc):
                ph = psum_t("ppe")
                js = slice(jc * 128, (jc + 1) * 128)
                for kb in range(4):
                    nc.tensor.matmul(ph[:, kb, :], khT[:, kb, js],
                                     qhT[:, b, :], start=True, stop=True)
                ex = exp_pool.tile([128, 4, S], bf16, tag="ex")
                nc.scalar.activation(ex[:], ph[:], AF.Exp)
                return ph, ex

            def emit_fen_part(b, jc, ph, ex):
                # fenmu into slot 0 (freed by the exp read), then 1/fenmu, rt
                for kb in range(4):
                    nc.tensor.matmul(ph[:, 0, :], id_s[:], ex[:, kb, :],
                                     start=(kb == 0), stop=(kb == 3))
                wr = wrp.tile([128, S], f32, tag="wr")
                nc.vector.reciprocal_approx_fast(wr[:], ph[:, 0, :])
                nc.gpsimd.tensor_tensor(rt_all[:, b, jc, :], ex[:, b, :],
                                        wr[:], op=OP.mult)

            # ---- P3 per b: scores + e/e2 + Z/Q (slots 1-2 of a P2 tile) ----
            def emit_p3(b, ph=None):
                ps3 = ph if ph is not None else psum_t("p3")
                for ic in range(4):
                    for jc in range(4):
                        nc.tensor.matmul(
                            ps3[:, 1 + ic // 2,
                                (ic % 2) * 256:(ic % 2 + 1) * 256],
                            rt_all[:, b, jc, ic * 128:(ic + 1) * 128],
                            vh_all[:, jc].rearrange("p c d -> p (c d)"),
                            start=(jc == 0), stop=(jc == 3),
                        )
                nc.scalar.activation(
                    e_all[:, b, 0, :].rearrange("p (a i) -> p a i", a=2),
                    ps3[:, 1:3, :], AF.Exp)
                e2eng = nc.vector if b < 2 else nc.gpsimd
                e2eng.tensor_tensor(e_all[:, b, 1, :], e_all[:, b, 0, :],
                                    e_all[:, b, 0, :], op=OP.mult)
                nc.vector.tensor_reduce(
                    Z_all[:, b * 16:(b + 1) * 16],
                    e_all[:, b, 0, :].rearrange("p (g d) -> p g d", d=DK),
                    axis=AX.X, op=OP.add)
                nc.vector.tensor_reduce(
                    Q_all[:, b * 16:(b + 1) * 16],
                    e_all[:, b, 1, :].rearrange("p (g d) -> p g d", d=DK),
                    axis=AX.X, op=OP.add)

            # ---- P4: stats per b-pair h (w1, w0) — DVE bit-trick rsqrt ----
            def emit_stats(h, ph=None):
                c0, c1 = h * 32, (h + 1) * 32
                Zs, Qs = Z_all[:, c0:c1], Q_all[:, c0:c1]
                t = stp.tile([128, 32], f32, tag="t", name="t")
                nc.vector.tensor_tensor(t[:], Zs, Zs, op=OP.mult)
                s = stp.tile([128, 32], f32, tag="s", name="s")
                nc.vector.scalar_tensor_tensor(
                    s[:], t[:], -1.0 / DK, Qs, op0=OP.mult, op1=OP.add)
                rinv = stp.tile([128, 32], f32, tag="rinv", name="rinv")
                nc.vector.reciprocal(rinv[:], t[:])
                v63 = stp.tile([128, 32], f32, tag="v63", name="v63")
                nc.vector.tensor_tensor(v63[:], s[:], rinv[:], op=OP.mult)
                r_ = stp.tile([128, 32], f32, tag="r_", name="r_")
                nc.vector.tensor_scalar(r_[:].bitcast(i32), v63[:].bitcast(i32),
                                        1, None, op0=OP.logical_shift_right)
                nc.vector.tensor_scalar(r_[:].bitcast(i32), r_[:].bitcast(i32),
                                        -1, 0x5F3759DF, op0=OP.mult, op1=OP.add)
                nt = stp.tile([128, 32], f32, tag="nt", name="nt")
                for _ in range(2):
                    nc.vector.tensor_tensor(nt[:], v63[:], r_[:], op=OP.mult)
                    nc.vector.tensor_tensor(nt[:], nt[:], r_[:], op=OP.mult)
                    nc.vector.tensor_scalar(nt[:], nt[:], -0.5, 1.5,
                                            op0=OP.mult, op1=OP.add)
                    nc.vector.tensor_tensor(r_[:], r_[:], nt[:], op=OP.mult)
                R_ = stp.tile([128, 32], f32, tag="R_", name="R_")
                nc.vector.tensor_scalar(R_[:], r_[:], float(np.sqrt(DK - 1.0)),
                                        None, op0=OP.mult)
                u_ = stp.tile([128, 32], f32, tag="u_", name="u_")
                nc.vector.tensor_scalar(u_[:], R_[:], -EPS, 1.0,
                                        op0=OP.mult, op1=OP.add)
                g = stp.tile([128, 32], f32, tag="g", name="g")
                nc.vector.tensor_tensor(g[:], R_[:], u_[:], op=OP.mult)
                zr = stp.tile([128, 32], f32, tag="zr", name="zr")
                nc.vector.reciprocal(zr[:], Zs)
                nc.vector.tensor_tensor(w1_all[:, c0:c1], g[:], zr[:],
                                        op=OP.mult)
                gs = stp.tile([128, 8], f32, tag="gs", name="gs")
                nc.vector.tensor_reduce(
                    gs[:], g[:].rearrange("p (s c) -> p s c", c=4), axis=AX.X,
                    op=OP.add)
                nc.vector.tensor_scalar(w0_all[:, h * 8:(h + 1) * 8], gs[:],
                                        -1.0 / DK, None, op0=OP.mult)
                slot = 3 if ph is not None else 0
                pw = ph if ph is not None else psum_t("pw")
                nc.tensor.matmul(pw[:8, slot, 0:128],
                                 w0_all[:, h * 8:(h + 1) * 8],
                                 idf_s[:], is_transpose=True, start=True,
                                 stop=True)
                w0Th = w0T0 if h == 0 else w0T1
                nc.vector.tensor_copy(w0Th[:, :], pw[:8, slot, 0:128])
                nc.sync.dma_start(
                    w0f[0:1, h * 1024:(h + 1) * 1024]
                    .rearrange("o (s f) -> o s f", s=8),
                    w0Th[:, :])

            # ---- P5 per b (elementwise prep + PE transpose/rank-1) ----
            def emit_p5_prep(b):
                w1e = p5p.tile([128, 16, DK], bf16, tag="w1e")
                nc.vector.tensor_copy(
                    w1e[:],
                    w1_all[:, b * 16:(b + 1) * 16].unsqueeze(-1)
                    .broadcast_to((128, 16, DK)))
                # contiguous 2x TT, then 3 strided slice-adds over c
                bsc = p5p.tile([128, 4, 4, DK], bf16, tag="bsc")
                nc.vector.tensor_tensor(
                    bsc[:].rearrange("p i c d -> p (i c d)"),
                    e_all[:, b, 0, :],
                    w1e[:].rearrange("p (i c) d -> p (i c d)", c=4),
                    op=OP.mult)
                eng = nc.vector if b < 2 else nc.gpsimd
                t01 = p5p.tile([128, 4, DK], bf16, tag="t01")
                eng.tensor_tensor(t01[:], bsc[:, :, 0, :], bsc[:, :, 1, :],
                                  op=OP.add)
                t23 = p5p.tile([128, 4, DK], bf16, tag="t23")
                eng.tensor_tensor(t23[:], bsc[:, :, 2, :], bsc[:, :, 3, :],
                                  op=OP.add)
                ball = p5p.tile([128, 4, DK], bf16, tag="ball")
                eng.tensor_tensor(ball[:], t01[:], t23[:], op=OP.add)
                return ball

            def emit_p5(b, ball):
                pbig = psum_t("p5")
                for ic in range(4):
                    nc.tensor.matmul(pbig[0:64, 0,
                                          ic * 128:(ic + 1) * 128],
                                     ball[:, ic, :], id_s[:],
                                     start=True, stop=False,
                                     skip_group_check=True)
                    slot = b * 4 + ic
                    nc.tensor.matmul(
                        pbig[0:64, 0, ic * 128:(ic + 1) * 128],
                        ones_f[:, 0:DK],
                        w0f[0:1, slot * 128:(slot + 1) * 128],
                        start=False, stop=True, skip_group_check=True,
                    )
                nc.vector.tensor_scalar(
                    hq[0:64, b, :], pbig[0:64, 0, :],
                    al_s[:], b4_s[:], op0=OP.mult, op1=OP.add,
                )

            # ---- P6 per b: out = WoP^T @ [heads; qhT] (+bo), copies on ACT
            def emit_p6(b):
                po = psum_t("p6")
                for nch in range(4):
                    nc.tensor.matmul(po[:, nch, :], WoP_s[:, nch, :],
                                     hq[:, b, :], start=True, stop=True)
                ot = otp.tile([128, 4, S], bf16, tag="ot")
                if bo_zero:
                    nc.scalar.activation(ot[:], po[:], AF.Identity)
                else:
                    for nch in range(4):
                        nc.scalar.activation(ot[:, nch, :], po[:, nch, :],
                                             AF.Identity,
                                             bias=bo2_s[:, nch:nch + 1])
                nc.sync.dma_start(
                    outT_d[b].rearrange("(n p) i -> p n i", p=128), ot[:])

            # ---- emission schedule ----
            for kb in range(4):
                emit_proj(Wk_s, bkc_s, ktiles[kb], khT[:, kb, :], True)
                if kb == 1:
                    # Exp table load in the ACT gap between k-proj copies
                    nc.scalar.activation(warm_z[0:1, 0:8], warm_z[0:1, 0:8],
                                         AF.Exp)
            emit_qproj(0)
            emit_bvb()

            # extras at loop index i borrow the (just consumed) PSUM tile of
            # unit i-1: vh/P3 use slots 1-2, q-proj/stats use slot 3 — so the
            # 2-buffer rotation parity of the E-unit stream is preserved.
            extras = {
                2: [lambda ph: emit_vh(0, ph)],
                3: [lambda ph: emit_qproj(1, ph)],
                4: [lambda ph: emit_vh(1, ph)],
                5: [lambda ph: emit_vh(2, ph)],
                6: [lambda ph: emit_vh(3, ph)],
                7: [lambda ph: emit_qproj(2, ph)],
                8: [lambda ph: emit_p3(0, ph)],
                11: [lambda ph: emit_qproj(3, ph),
                     lambda ph: emit_p3(1, ph)],
                13: [lambda ph: emit_stats(0, ph),
                     lambda ph: emit_p3(2, ph)],
            }
            units = [(b, jc) for b in range(BS) for jc in range(4)]
            pend = None
            for i, (b, jc) in enumerate(units):
                ph, ex = emit_e_part(b, jc)
                if pend is not None:
                    emit_fen_part(*pend)
                    for fn in extras.get(i, []):
                        fn(pend[2])
                pend = (b, jc, ph, ex)
            emit_fen_part(*pend)

            ball0 = emit_p5_prep(0)
            emit_p3(3)
            emit_p5(0, ball0)
            emit_p6(0)
            ball1 = emit_p5_prep(1)
            emit_p5(1, ball1)
            emit_p6(1)
            emit_stats(1)
            ball2 = emit_p5_prep(2)
            emit_p5(2, ball2)
            emit_p6(2)
            ball3 = emit_p5_prep(3)
            emit_p5(3, ball3)
            emit_p6(3)

    return nc


def _build(bo_zero):
    import concourse.bass as bass  # noqa
    import concourse.tile as tile
    from concourse import bacc, mybir

    nc = bacc.Bacc("TRN2", target_bir_lowering=False, debug=False,
                   num_devices=NCORES)
    build_program(nc, tile, mybir, bo_zero)
    nc.compile()
    return nc


_cached_nc = None
_cached_bo_zero = None


def make_in_maps(q, k, v, Wq, bq, Wk, bk, Wv, bv, Wo, bo, alpha, beta):
    import ml_dtypes
    bft = ml_dtypes.bfloat16

    def prelay(x):
        # [S, DM] per batch -> transposed [DM, S] -> [128, 4, S] layout
        xT = np.swapaxes(np.asarray(x, np.float32), 1, 2)  # [B, DM, S]
        return np.ascontiguousarray(
            xT.reshape(BS, 4, 128, S).transpose(0, 2, 1, 3)).astype(bft)

    def wlay(W):  # [DM, DK] -> [128, 4, DK]
        return np.ascontiguousarray(
            np.asarray(W, np.float32).reshape(4, 128, DK).transpose(1, 0, 2)
        ).astype(bft)

    qT, kT, vT = prelay(q), prelay(k), prelay(v)
    Wq, Wk, Wv, Wo = (np.asarray(x, np.float32) for x in (Wq, Wk, Wv, Wo))
    bq, bk, bv, bo = (np.asarray(x, np.float32) for x in (bq, bk, bv, bo))
    alpha, beta = np.asarray(alpha, np.float32), np.asarray(beta, np.float32)
    ident = np.eye(128, dtype=ml_dtypes.bfloat16)
    identf = np.eye(128, dtype=np.float32)
    scale = np.float32(1.0 / np.sqrt(np.float32(DK)))  # fenmu sqrt(DK) -> Wv
    in_maps = []
    for h in range(NCORES):
        sl = slice(h * DK, (h + 1) * DK)
        WoP = np.zeros((128, 4, 128), np.float32)
        for nch in range(4):
            WoP[0:64, nch, :] = Wo[sl, nch * 128:(nch + 1) * 128]
            WoP[64:128, nch, :] = 4.0 * Wo[sl, nch * 128:(nch + 1) * 128]
        in_maps.append({
            "qT": qT, "kT": kT, "vT": vT,
            "Wq": wlay(Wq[:, sl]),
            "Wk": wlay(Wk[:, sl]),
            "Wv": wlay(Wv[:, sl] * scale),
            "bqc": np.ascontiguousarray(bq[sl])[:, None].astype(np.float32),
            "bkc": np.ascontiguousarray(bk[sl])[:, None].astype(np.float32),
            "bv": np.ascontiguousarray(bv[sl] * scale)[None, :].astype(bft),
            "WoP": WoP.astype(bft),
            "bo2": np.ascontiguousarray(
                (bo if h == 0 else np.zeros_like(bo)).reshape(4, 128).T
            ).astype(np.float32),
            "alpha": np.ascontiguousarray(alpha)[:, None],
            "beta4": np.ascontiguousarray(4.0 * beta)[:, None],
            "ident": ident, "identf": identf,
        })
    return in_maps


def assemble(results):
    out = np.zeros((BS, S, DM), np.float32)
    for r in results:
        out += np.swapaxes(np.asarray(r["outT"], np.float32), 1, 2)
    return out


def kernel(**inputs) -> np.ndarray:
    global _cached_nc, _cached_bo_zero
    from concourse.bass_utils import run_bass_kernel_spmd

    bo_zero = bool(np.all(np.asarray(inputs["bo"]) == 0.0))
    if _cached_nc is None or _cached_bo_zero != bo_zero:
        _cached_nc = _build(bo_zero)
        _cached_bo_zero = bo_zero
    in_maps = make_in_maps(**inputs)
    res = run_bass_kernel_spmd(_cached_nc, in_maps, list(range(NCORES)))
    return assemble(res.results)
